# revision 18
# baseline (speedup 1.0000x reference)
"""Trainium2 Bass kernel for Graph_Attention_Union (gnn_message_passing).

Data-parallel over batch: B=32 sharded as 4 samples per core x 8 cores.
All compute per-sample stays on one core; no collectives.

v3 design (mixed precision, measured against a 2e-2 rel-err budget):
 - Self-attention scores are diagonal-dominated (S_nn = |q_n|^2 ~ 26..44)
   so exp(S) spans e^-14..e^44: E must be bf16 (fp8 overflows -> 17% err).
   Quantizing the xfg path or the final-conv operands to fp8 adds ~4% err
   each, so all value/projection/final matmuls run bf16 (1 PE cycle/row).
 - The ONE affordable fp8 step (+0.6% err): storing q and zt in fp8e4 and
   computing the score matmuls with MatmulPerfMode.DoubleRow (K=256 packed,
   0.5 cycles/row). q/zt feed nothing but scores.
 - Self-branch Z: accum_out on the exp activations ([128,NT] row sums),
   small DVE reciprocal, identity-scaled diag tiles (Pool) and a ones
   matmul broadcast (baseline-proven pattern).
 - z-branch attention is computed TRANSPOSED (S_z^T [49-on-partitions, NX])
   so its exp is one activation instruction; Z_z comes replicated across
   partitions from an all-ones stationary matmul and reciprocal_approx_fast
   turns it into the broadcast invZ tile directly (plain reciprocal on
   [128,961] costs 6.2us -- approx_fast is ~5x faster at 18 bits).
 - Engine split: ACT = q-bias-cast + all exps (+accum) ; DVE = xfg/xfgp
   relus, norm-muls, reciprocals, final relu; Pool = diag tiles, memsets.
"""

import sys

for _p in ("/opt/trn_rl_repo",):
    if _p not in sys.path:
        sys.path.insert(0, _p)

import numpy as np

from concourse import bacc, bass, masks, mybir
from concourse.bass_utils import run_bass_kernel_spmd
from concourse.tile import TileContext

FP = mybir.dt.float32
BF = mybir.dt.bfloat16
F8 = mybir.dt.float8e4
AF = mybir.ActivationFunctionType
DR = mybir.MatmulPerfMode.DoubleRow

B, C, O = 32, 256, 256
HZ, WZ, HX, WX = 7, 7, 31, 31
NZ, NX = HZ * WZ, HX * WX  # 49, 961
NCORES = 8
BL = B // NCORES  # 4 samples per core
EPS = 1e-5

KT = C // 128           # 2 k-planes over channels
NT = (NX + 127) // 128  # 8 m-tiles over Nx (7*128 + 65)
LAST = NX - 7 * 128     # 65
FKT = 3 * C // 128      # 6 k-planes for final conv
NZP = 64                # zt stationary padded to 64 cols for DoubleRow

# free-dim chunks of NX (<=512 moving rows per matmul)
CHUNKS = [(0, 512), (512, NX - 512)]


def build(nonzero_bg: bool, nonzero_fib: bool):
    nc = bacc.Bacc(None, target_bir_lowering=False)

    xf_d = nc.declare_dram_parameter("xf", [BL, C, NX], BF, isOutput=False)
    zf_d = nc.declare_dram_parameter("zf", [BL, C, NZ], BF, isOutput=False)
    wq_d = nc.declare_dram_parameter("wqT", [C, C], BF, isOutput=False)
    ws_d = nc.declare_dram_parameter("wsT", [C, C], BF, isOutput=False)
    wg_d = nc.declare_dram_parameter("wgT", [C, C], BF, isOutput=False)
    wfi_d = nc.declare_dram_parameter("wfiT", [3 * C, O], BF, isOutput=False)
    vec_d = nc.declare_dram_parameter("vecs", [5, 2, 128], FP, isOutput=False)
    out_d = nc.declare_dram_parameter("out", [BL, O, NX], FP, isOutput=True)

    with TileContext(nc) as tc:
        with (
            tc.tile_pool(name="const", bufs=1) as constp,
            tc.tile_pool(name="io", bufs=2) as iop,
            tc.tile_pool(name="work", bufs=2) as wkp,
            tc.tile_pool(name="zbat", bufs=1) as zbp,
            tc.tile_pool(name="psbig", bufs=3, space="PSUM") as psb,
            tc.tile_pool(name="pssmall", bufs=2, space="PSUM") as pss,
        ):
            # ---- constants ----
            wq_sb = constp.tile([128, KT, C], BF)
            ws_sb = constp.tile([128, KT, C], BF)
            wg_sb = constp.tile([128, KT, C], BF)
            wfi_sb = constp.tile([128, FKT, O], BF)
            for k in range(KT):
                nc.sync.dma_start(wq_sb[:, k, :], wq_d[k * 128:(k + 1) * 128, :])
                nc.sync.dma_start(ws_sb[:, k, :], ws_d[k * 128:(k + 1) * 128, :])
                nc.sync.dma_start(wg_sb[:, k, :], wg_d[k * 128:(k + 1) * 128, :])
            for k in range(FKT):
                nc.sync.dma_start(wfi_sb[:, k, :], wfi_d[k * 128:(k + 1) * 128, :])
            vecs = constp.tile([128, 5, 2], FP)
            nc.sync.dma_start(vecs[:], vec_d.rearrange("v t p -> p v t"))
            bq = [vecs[:, 0, t:t + 1] for t in range(2)]
            bs = [vecs[:, 1, t:t + 1] for t in range(2)]
            bg = [vecs[:, 2, t:t + 1] for t in range(2)]
            fis = [vecs[:, 3, t:t + 1] for t in range(2)]
            fib = [vecs[:, 4, t:t + 1] for t in range(2)]
            bg_row = constp.tile([1, C], FP)
            nc.gpsimd.dma_start(bg_row[:], vec_d[2:3].rearrange("o t p -> o (t p)"))
            ones_row = constp.tile([1, 128], FP)
            nc.vector.memset(ones_row[:], 1.0)
            ones128 = constp.tile([128, 128], BF)
            nc.vector.memset(ones128[:], 1.0)
            ident = constp.tile([128, 128], BF)
            masks.make_identity(nc, ident[:])

            # ---- z-branch convs, batched over all samples (tiny) ----
            zf_sb = zbp.tile([128, KT, BL, NZ], BF)
            for k in range(KT):
                nc.sync.dma_start(
                    zf_sb[:, k, :, :],
                    zf_d[:, k * 128:(k + 1) * 128, :].rearrange("s c n -> c s n"))
            # zt: fp8, padded to 64 cols per sample for the DR score matmul
            zt_sb = zbp.tile([128, KT, BL, NZP], F8)
            nc.gpsimd.memset(zt_sb[:, :, :, NZ:], 0.0)
            zg_sb = zbp.tile([128, KT, BL, NZ], BF)
            for oi in range(KT):
                psz = pss.tile([128, BL, NZ], FP, tag="small")
                for k in range(KT):
                    nc.tensor.matmul(
                        psz[:].rearrange("p s n -> p (s n)"),
                        ws_sb[:, k, oi * 128:(oi + 1) * 128],
                        zf_sb[:, k, :, :].rearrange("p s n -> p (s n)"),
                        start=(k == 0), stop=(k == KT - 1))
                nc.scalar.activation(
                    zt_sb[:, oi, :, :NZ], psz[:],
                    AF.Identity, bias=bs[oi])
                psz2 = pss.tile([128, BL * NZ], FP, tag="small")
                for k in range(KT):
                    nc.tensor.matmul(
                        psz2[:], wg_sb[:, k, oi * 128:(oi + 1) * 128],
                        zf_sb[:, k, :, :].rearrange("p s n -> p (s n)"),
                        start=(k == 0), stop=(k == KT - 1))
                nc.scalar.activation(
                    zg_sb[:, oi, :, :].rearrange("p s n -> p (s n)"), psz2[:],
                    AF.Relu, bias=bg[oi])
            # zgp [m=49, s, c] = zg^T
            zgp_sb = zbp.tile([NZ, BL, C], BF)
            for s in range(BL):
                for oi in range(KT):
                    pst = pss.tile([NZ, 128], BF, tag="small")
                    nc.tensor.transpose(pst[:], zg_sb[:, oi, s, :], ident[:])
                    nc.vector.tensor_copy(zgp_sb[:, s, oi * 128:(oi + 1) * 128], pst[:])

            for s in range(BL):
                # ---- load xf ----
                xf_sb = iop.tile([128, KT, NX], BF)
                for k in range(KT):
                    nc.sync.dma_start(xf_sb[:, k, :],
                                      xf_d[s, k * 128:(k + 1) * 128, :])

                # ---- q: bf16 conv -> fp8 store (padded cols for DR) ----
                q_sb = wkp.tile([128, KT, NT * 128], F8)
                nc.gpsimd.memset(q_sb[:, :, NX:], 0.0)
                for oi in range(KT):
                    psq = psb.tile([128, NX], FP, tag="big")
                    for k in range(KT):
                        for (c0, cn) in CHUNKS:
                            nc.tensor.matmul(
                                psq[:, c0:c0 + cn],
                                wq_sb[:, k, oi * 128:(oi + 1) * 128],
                                xf_sb[:, k, c0:c0 + cn],
                                start=(k == 0), stop=(k == KT - 1))
                    nc.scalar.activation(q_sb[:, oi, :NX], psq[:], AF.Identity,
                                         bias=bq[oi])

                # ---- xfg: bf16 conv, relu (DVE) ----
                xfg_sb = wkp.tile([128, KT, NX], BF)
                for oi in range(KT):
                    psg = psb.tile([128, NX], FP, tag="big")
                    for k in range(KT):
                        for (c0, cn) in CHUNKS:
                            nc.tensor.matmul(
                                psg[:, c0:c0 + cn],
                                wg_sb[:, k, oi * 128:(oi + 1) * 128],
                                xf_sb[:, k, c0:c0 + cn],
                                start=(k == 0), stop=(k == KT - 1))
                    nc.vector.tensor_scalar(
                        xfg_sb[:, oi, :], psg[:], bg[oi], 0.0,
                        mybir.AluOpType.add, mybir.AluOpType.max)

                # ---- xfgp [n, c]: duplicate conv, relu (DVE) ----
                xfgp_sb = wkp.tile([128, NT, C], BF)
                for mi in range(NT):
                    mw = 128 if mi < NT - 1 else LAST
                    psp = pss.tile([128, C], FP, tag="small")
                    for k in range(KT):
                        nc.tensor.matmul(
                            psp[:mw, :],
                            xf_sb[:, k, mi * 128:mi * 128 + mw],
                            wg_sb[:, k, :],
                            start=(k == 0),
                            stop=(k == KT - 1) and not nonzero_bg)
                    if nonzero_bg:
                        nc.tensor.matmul(psp[:mw, :], ones_row[:, :mw], bg_row[:],
                                         start=False, stop=True)
                    nc.vector.tensor_scalar_max(xfgp_sb[:mw, mi, :], psp[:mw, :], 0.0)

                # ---- z attention, transposed: S_z^T (DR fp8, 64 padded) ----
                ezt_sb = wkp.tile([NZ, NX], BF)
                pszt = psb.tile([NZP, NX], FP, tag="big")
                for (c0, cn) in CHUNKS:
                    nc.tensor.matmul(
                        pszt[:, c0:c0 + cn],
                        zt_sb[:, :, s, :],
                        q_sb[:, :, c0:c0 + cn],
                        start=True, stop=True, perf_mode=DR)
                nc.scalar.activation(ezt_sb[:], pszt[:NZ, :], AF.Exp)

                # ---- self attention: S = q^T q (DR fp8), E = exp(S) bf16 ----
                e_sb = wkp.tile([128, NT, NX], BF)
                zs_sb = wkp.tile([128, NT], FP)
                nc.vector.memset(zs_sb[:], 1.0)
                for mi in range(NT):
                    mw = 128 if mi < NT - 1 else LAST
                    pss_t = psb.tile([128, NX], FP, tag="big")
                    for (c0, cn) in CHUNKS:
                        nc.tensor.matmul(
                            pss_t[:, c0:c0 + cn],
                            q_sb[:, :, mi * 128:(mi + 1) * 128],
                            q_sb[:, :, c0:c0 + cn],
                            start=True, stop=True, perf_mode=DR)
                    nc.scalar.activation(e_sb[:mw, mi, :], pss_t[:mw, :], AF.Exp,
                                         accum_out=zs_sb[:mw, mi:mi + 1])

                # ---- z tail: Z_z replicated + fast recip -> invZ bcast ----
                psZz = psb.tile([128, NX], FP, tag="big")
                for (c0, cn) in CHUNKS:
                    nc.tensor.matmul(psZz[:, c0:c0 + cn], ones128[:NZ, :],
                                     ezt_sb[:, c0:c0 + cn], start=True, stop=True)
                izz_sb = wkp.tile([128, NX], FP)
                nc.vector.reciprocal_approx_fast(out=izz_sb[:], in_=psZz[:])
                xemb_sb = wkp.tile([128, KT, NX], BF)
                for oi in range(KT):
                    pse = psb.tile([128, NX], FP, tag="big")
                    for (c0, cn) in CHUNKS:
                        nc.tensor.matmul(pse[:, c0:c0 + cn],
                                         zgp_sb[:, s, oi * 128:(oi + 1) * 128],
                                         ezt_sb[:, c0:c0 + cn],
                                         start=True, stop=True)
                    nc.vector.tensor_mul(xemb_sb[:, oi, :], pse[:], izz_sb[:])

                # ---- self Z: invZ broadcast via diag tiles + ones matmul ----
                izs_sb = wkp.tile([128, NT], FP)
                nc.vector.reciprocal(izs_sb[:], zs_sb[:])
                diag_sb = wkp.tile([128, NT, 128], BF)
                for mi in range(NT):
                    nc.gpsimd.tensor_scalar_mul(diag_sb[:, mi, :], ident[:],
                                                izs_sb[:, mi:mi + 1])
                psbc = psb.tile([128, NX], FP, tag="big")
                dflat = diag_sb[:].rearrange("p a b -> p (a b)")
                nc.tensor.matmul(psbc[:, 0:512], ones128[:], dflat[:, 0:512],
                                 start=True, stop=True)
                nc.tensor.matmul(psbc[:, 512:NX], ones128[:], dflat[:, 512:NX],
                                 start=True, stop=True)
                bcast_sb = wkp.tile([128, NX], FP)
                nc.vector.tensor_copy(bcast_sb[:], psbc[:])

                # ---- self emb [c, n] = xfgp^T @ E, normalized on drain ----
                xself_sb = wkp.tile([128, KT, NX], BF)
                for oi in range(KT):
                    psu = psb.tile([128, NX], FP, tag="big")
                    for k in range(NT):
                        kw = 128 if k < NT - 1 else LAST
                        for (c0, cn) in CHUNKS:
                            nc.tensor.matmul(
                                psu[:, c0:c0 + cn],
                                xfgp_sb[:kw, k, oi * 128:(oi + 1) * 128],
                                e_sb[:kw, k, c0:c0 + cn],
                                start=(k == 0), stop=(k == NT - 1))
                    nc.vector.tensor_mul(xself_sb[:, oi, :], psu[:], bcast_sb[:])

                # ---- final conv: out = relu(fis*(Wfi @ [emb; self; xfg]) + fib) ----
                xcat = [xemb_sb, xself_sb, xfg_sb]
                out_sb = iop.tile([128, KT, NX], FP)
                for oi in range(KT):
                    psf = psb.tile([128, NX], FP, tag="big")
                    for k in range(FKT):
                        sec, kk = divmod(k, KT)
                        for (c0, cn) in CHUNKS:
                            nc.tensor.matmul(
                                psf[:, c0:c0 + cn],
                                wfi_sb[:, k, oi * 128:(oi + 1) * 128],
                                xcat[sec][:, kk, c0:c0 + cn],
                                start=(k == 0), stop=(k == FKT - 1))
                    if nonzero_fib:
                        nc.scalar.activation(out_sb[:, oi, :], psf[:], AF.Relu,
                                             bias=fib[oi], scale=fis[oi])
                    else:
                        nc.vector.tensor_scalar(
                            out_sb[:, oi, :], psf[:], fis[oi], 0.0,
                            mybir.AluOpType.mult, mybir.AluOpType.max)
                for oi in range(KT):
                    nc.sync.dma_start(out_d[s, oi * 128:(oi + 1) * 128, :],
                                      out_sb[:, oi, :])

    nc.compile()
    return nc


_NC_CACHE = {}


def kernel(**inputs):
    xf = np.ascontiguousarray(inputs["xf"], dtype=np.float32).reshape(B, C, NX)
    zf = np.ascontiguousarray(inputs["zf"], dtype=np.float32).reshape(B, C, NZ)
    Wq = np.asarray(inputs["Wq"], dtype=np.float32)
    bq_v = np.asarray(inputs["bq"], dtype=np.float32)
    Ws = np.asarray(inputs["Ws"], dtype=np.float32)
    bs_v = np.asarray(inputs["bs"], dtype=np.float32)
    Wg = np.asarray(inputs["Wg"], dtype=np.float32)
    bg_v = np.asarray(inputs["bg"], dtype=np.float32)

    g_s = inputs["g_gamma"].astype(np.float32) / np.sqrt(inputs["g_var"].astype(np.float32) + EPS)
    g_b = (bg_v - inputs["g_mean"].astype(np.float32)) * g_s + inputs["g_beta"].astype(np.float32)
    Wg_eff = (g_s[:, None] * Wg).astype(np.float32)

    fi_s = inputs["fi_gamma"].astype(np.float32) / np.sqrt(inputs["fi_var"].astype(np.float32) + EPS)
    fi_b = ((inputs["bfi"].astype(np.float32) - inputs["fi_mean"].astype(np.float32)) * fi_s
            + inputs["fi_beta"].astype(np.float32))
    Wfi = np.asarray(inputs["Wfi"], dtype=np.float32)

    vecs = np.stack([bq_v, bs_v, g_b, fi_s, fi_b]).reshape(5, 2, 128).astype(np.float32)
    nonzero_bg = bool(np.any(g_b != 0.0))
    nonzero_fib = bool(np.any(fi_b != 0.0))

    key = (nonzero_bg, nonzero_fib)
    if key not in _NC_CACHE:
        _NC_CACHE[key] = build(*key)
    nc = _NC_CACHE[key]

    import ml_dtypes
    bf16 = ml_dtypes.bfloat16
    wqT = np.ascontiguousarray(Wq.T).astype(bf16)
    wsT = np.ascontiguousarray(Ws.T).astype(bf16)
    wgT = np.ascontiguousarray(Wg_eff.T).astype(bf16)
    wfiT = np.ascontiguousarray(Wfi.T).astype(bf16)
    xf_b = xf.astype(bf16)
    zf_b = zf.astype(bf16)

    in_maps = []
    for i in range(NCORES):
        in_maps.append({
            "xf": np.ascontiguousarray(xf_b[i * BL:(i + 1) * BL]),
            "zf": np.ascontiguousarray(zf_b[i * BL:(i + 1) * BL]),
            "wqT": wqT, "wsT": wsT, "wgT": wgT, "wfiT": wfiT,
            "vecs": vecs,
        })

    import os
    trace = os.environ.get("BASS_KERNEL_TRACE", "0") == "1"
    res = run_bass_kernel_spmd(nc, in_maps, list(range(NCORES)), trace=trace)
    LAST_RUN["exec_time_ns"] = res.exec_time_ns
    if res.instructions_and_trace is not None:
        LAST_RUN["trace_path"] = res.instructions_and_trace[1]
    LAST_RUN["profile_json"] = res.profile_json
    out = np.concatenate([r["out"] for r in res.results], axis=0)
    return out.reshape(B, O, HX, WX).astype(np.float32)


LAST_RUN = {}


if __name__ == "__main__":
    rng = np.random.default_rng(0)
    demo = {
        "zf": rng.standard_normal((B, C, HZ, WZ), dtype=np.float32),
        "xf": rng.standard_normal((B, C, HX, WX), dtype=np.float32),
        "Wq": rng.standard_normal((C, C), dtype=np.float32) * 0.02,
        "bq": np.zeros(C, np.float32),
        "Ws": rng.standard_normal((C, C), dtype=np.float32) * 0.02,
        "bs": np.zeros(C, np.float32),
        "Wg": rng.standard_normal((C, C), dtype=np.float32) * 0.02,
        "bg": np.zeros(C, np.float32),
        "g_gamma": np.ones(C, np.float32), "g_beta": np.zeros(C, np.float32),
        "g_mean": np.zeros(C, np.float32), "g_var": np.ones(C, np.float32),
        "Wfi": rng.standard_normal((O, 3 * C), dtype=np.float32) * 0.02,
        "bfi": np.zeros(O, np.float32),
        "fi_gamma": np.ones(O, np.float32), "fi_beta": np.zeros(O, np.float32),
        "fi_mean": np.zeros(O, np.float32), "fi_var": np.ones(O, np.float32),
    }
    print(kernel(**demo).shape)


# revision 21
# speedup vs baseline: 1.5329x; 1.5329x over previous
"""Trainium2 Bass kernel for Graph_Attention_Union (gnn_message_passing).

Data-parallel over batch: B=32 sharded as 4 samples per core x 8 cores.
All compute per-sample stays on one core; no collectives.

v3 design (mixed precision, measured against a 2e-2 rel-err budget):
 - Self-attention scores are diagonal-dominated (S_nn = |q_n|^2 ~ 26..44)
   so exp(S) spans e^-14..e^44: E must be bf16 (fp8 overflows -> 17% err).
   Quantizing the xfg path or the final-conv operands to fp8 adds ~4% err
   each, so all value/projection/final matmuls run bf16 (1 PE cycle/row).
 - The ONE affordable fp8 step (+0.6% err): storing q and zt in fp8e4 and
   computing the score matmuls with MatmulPerfMode.DoubleRow (K=256 packed,
   0.5 cycles/row). q/zt feed nothing but scores.
 - Self-branch Z: accum_out on the exp activations ([128,NT] row sums),
   small DVE reciprocal, identity-scaled diag tiles (Pool) and a ones
   matmul broadcast (baseline-proven pattern).
 - z-branch attention is computed TRANSPOSED (S_z^T [49-on-partitions, NX])
   so its exp is one activation instruction; Z_z comes replicated across
   partitions from an all-ones stationary matmul and reciprocal_approx_fast
   turns it into the broadcast invZ tile directly (plain reciprocal on
   [128,961] costs 6.2us -- approx_fast is ~5x faster at 18 bits).
 - Engine split: ACT = q-bias-cast + all exps (+accum) ; DVE = xfg/xfgp
   relus, norm-muls, reciprocals, final relu; Pool = diag tiles, memsets.
"""

import sys

for _p in ("/opt/trn_rl_repo",):
    if _p not in sys.path:
        sys.path.insert(0, _p)

import numpy as np

from concourse import bacc, bass, masks, mybir
from concourse.bass_utils import run_bass_kernel_spmd
from concourse.tile import TileContext

FP = mybir.dt.float32
BF = mybir.dt.bfloat16
F8 = mybir.dt.float8e4
AF = mybir.ActivationFunctionType
DR = mybir.MatmulPerfMode.DoubleRow

B, C, O = 32, 256, 256
HZ, WZ, HX, WX = 7, 7, 31, 31
NZ, NX = HZ * WZ, HX * WX  # 49, 961
NCORES = 8
BL = B // NCORES  # 4 samples per core
EPS = 1e-5

KT = C // 128           # 2 k-planes over channels
NT = (NX + 127) // 128  # 8 m-tiles over Nx (7*128 + 65)
LAST = NX - 7 * 128     # 65
FKT = 3 * C // 128      # 6 k-planes for final conv
NZP = 64                # zt stationary padded to 64 cols for DoubleRow

# free-dim chunks of NX (<=512 moving rows per matmul)
CHUNKS = [(0, 512), (512, NX - 512)]


def build(nonzero_bg: bool, nonzero_fib: bool):
    nc = bacc.Bacc(None, target_bir_lowering=False)

    xf_d = nc.declare_dram_parameter("xf", [BL, C, NX], BF, isOutput=False)
    zf_d = nc.declare_dram_parameter("zf", [BL, C, NZ], BF, isOutput=False)
    wq_d = nc.declare_dram_parameter("wqT", [C, C], BF, isOutput=False)
    ws_d = nc.declare_dram_parameter("wsT", [C, C], BF, isOutput=False)
    wg_d = nc.declare_dram_parameter("wgT", [C, C], BF, isOutput=False)
    wfi_d = nc.declare_dram_parameter("wfiT", [3 * C, O], BF, isOutput=False)
    vec_d = nc.declare_dram_parameter("vecs", [5, 2, 128], FP, isOutput=False)
    out_d = nc.declare_dram_parameter("out", [BL, O, NX], FP, isOutput=True)

    with TileContext(nc) as tc:
        with (
            tc.tile_pool(name="const", bufs=1) as constp,
            tc.tile_pool(name="io", bufs=2) as iop,
            tc.tile_pool(name="work", bufs=2) as wkp,
            tc.tile_pool(name="zbat", bufs=1) as zbp,
            tc.tile_pool(name="psbig", bufs=3, space="PSUM") as psb,
            tc.tile_pool(name="pssmall", bufs=2, space="PSUM") as pss,
        ):
            # ---- constants ----
            wq_sb = constp.tile([128, KT, C], BF)
            ws_sb = constp.tile([128, KT, C], BF)
            wg_sb = constp.tile([128, KT, C], BF)
            wfi_sb = constp.tile([128, FKT, O], BF)
            for k in range(KT):
                nc.scalar.dma_start(wq_sb[:, k, :], wq_d[k * 128:(k + 1) * 128, :])
                nc.sync.dma_start(ws_sb[:, k, :], ws_d[k * 128:(k + 1) * 128, :])
                nc.gpsimd.dma_start(wg_sb[:, k, :], wg_d[k * 128:(k + 1) * 128, :])
            for k in range(FKT):
                (nc.sync if k % 2 else nc.scalar).dma_start(
                    wfi_sb[:, k, :], wfi_d[k * 128:(k + 1) * 128, :])
            vecs = constp.tile([128, 5, 2], FP)
            nc.sync.dma_start(vecs[:], vec_d.rearrange("v t p -> p v t"))
            bq = [vecs[:, 0, t:t + 1] for t in range(2)]
            bs = [vecs[:, 1, t:t + 1] for t in range(2)]
            bg = [vecs[:, 2, t:t + 1] for t in range(2)]
            fis = [vecs[:, 3, t:t + 1] for t in range(2)]
            fib = [vecs[:, 4, t:t + 1] for t in range(2)]
            bg_row = constp.tile([1, C], FP)
            nc.gpsimd.dma_start(bg_row[:], vec_d[2:3].rearrange("o t p -> o (t p)"))
            ones_row = constp.tile([1, 128], FP)
            nc.vector.memset(ones_row[:], 1.0)
            ones128 = constp.tile([128, 128], BF)
            nc.vector.memset(ones128[:], 1.0)
            ident = constp.tile([128, 128], BF)
            masks.make_identity(nc, ident[:])

            # ---- z-branch convs, batched over all samples (tiny) ----
            zf_sb = zbp.tile([128, KT, BL, NZ], BF)
            for k in range(KT):
                nc.sync.dma_start(
                    zf_sb[:, k, :, :],
                    zf_d[:, k * 128:(k + 1) * 128, :].rearrange("s c n -> c s n"))
            # zt: fp8, padded to 64 cols per sample for the DR score matmul
            zt_sb = zbp.tile([128, KT, BL, NZP], F8)
            nc.gpsimd.memset(zt_sb[:, :, :, NZ:], 0.0)
            zg_sb = zbp.tile([128, KT, BL, NZ], BF)
            for oi in range(KT):
                psz = pss.tile([128, BL, NZ], FP, tag="small")
                for k in range(KT):
                    nc.tensor.matmul(
                        psz[:].rearrange("p s n -> p (s n)"),
                        ws_sb[:, k, oi * 128:(oi + 1) * 128],
                        zf_sb[:, k, :, :].rearrange("p s n -> p (s n)"),
                        start=(k == 0), stop=(k == KT - 1))
                nc.scalar.activation(
                    zt_sb[:, oi, :, :NZ], psz[:],
                    AF.Identity, bias=bs[oi])
                psz2 = pss.tile([128, BL * NZ], FP, tag="small")
                for k in range(KT):
                    nc.tensor.matmul(
                        psz2[:], wg_sb[:, k, oi * 128:(oi + 1) * 128],
                        zf_sb[:, k, :, :].rearrange("p s n -> p (s n)"),
                        start=(k == 0), stop=(k == KT - 1))
                nc.scalar.activation(
                    zg_sb[:, oi, :, :].rearrange("p s n -> p (s n)"), psz2[:],
                    AF.Relu, bias=bg[oi])
            # zgp [m=49, s, c] = zg^T
            zgp_sb = zbp.tile([NZ, BL, C], BF)
            for s in range(BL):
                for oi in range(KT):
                    pst = pss.tile([NZ, 128], BF, tag="small")
                    nc.tensor.transpose(pst[:], zg_sb[:, oi, s, :], ident[:])
                    nc.vector.tensor_copy(zgp_sb[:, s, oi * 128:(oi + 1) * 128], pst[:])

            ctx = {}

            def phase1(s):
                # loads + projections + scores + exps
                xf_sb = iop.tile([128, KT, NX], BF)
                for k in range(KT):
                    nc.sync.dma_start(xf_sb[:, k, :],
                                      xf_d[s, k * 128:(k + 1) * 128, :])

                # q: bf16 conv -> fp8 store (padded cols for DR)
                q_sb = wkp.tile([128, KT, NT * 128], F8)
                nc.gpsimd.memset(q_sb[:, :, NX:], 0.0)
                for oi in range(KT):
                    psq = psb.tile([128, NX], FP, tag="big")
                    for k in range(KT):
                        for (c0, cn) in CHUNKS:
                            nc.tensor.matmul(
                                psq[:, c0:c0 + cn],
                                wq_sb[:, k, oi * 128:(oi + 1) * 128],
                                xf_sb[:, k, c0:c0 + cn],
                                start=(k == 0), stop=(k == KT - 1))
                    nc.scalar.activation(q_sb[:, oi, :NX], psq[:], AF.Identity,
                                         bias=bq[oi])

                # xfg: bf16 conv, relu (DVE)
                xfg_sb = wkp.tile([128, KT, NX], BF)
                for oi in range(KT):
                    psg = psb.tile([128, NX], FP, tag="big")
                    for k in range(KT):
                        for (c0, cn) in CHUNKS:
                            nc.tensor.matmul(
                                psg[:, c0:c0 + cn],
                                wg_sb[:, k, oi * 128:(oi + 1) * 128],
                                xf_sb[:, k, c0:c0 + cn],
                                start=(k == 0), stop=(k == KT - 1))
                    nc.vector.tensor_scalar(
                        xfg_sb[:, oi, :], psg[:], bg[oi], 0.0,
                        mybir.AluOpType.add, mybir.AluOpType.max)

                # xfgp [n, c]: duplicate conv, relu (DVE)
                xfgp_sb = wkp.tile([128, NT, C], BF)
                for mi in range(NT):
                    mw = 128 if mi < NT - 1 else LAST
                    psp = pss.tile([128, C], FP, tag="small")
                    for k in range(KT):
                        nc.tensor.matmul(
                            psp[:mw, :],
                            xf_sb[:, k, mi * 128:mi * 128 + mw],
                            wg_sb[:, k, :],
                            start=(k == 0),
                            stop=(k == KT - 1) and not nonzero_bg)
                    if nonzero_bg:
                        nc.tensor.matmul(psp[:mw, :], ones_row[:, :mw], bg_row[:],
                                         start=False, stop=True)
                    nc.vector.tensor_scalar_max(xfgp_sb[:mw, mi, :], psp[:mw, :], 0.0)

                # z attention, transposed: S_z^T (DR fp8, 64 padded)
                ezt_sb = wkp.tile([NZ, NX], BF)
                pszt = psb.tile([NZP, NX], FP, tag="big")
                for (c0, cn) in CHUNKS:
                    nc.tensor.matmul(
                        pszt[:, c0:c0 + cn],
                        zt_sb[:, :, s, :],
                        q_sb[:, :, c0:c0 + cn],
                        start=True, stop=True, perf_mode=DR)
                nc.scalar.activation(ezt_sb[:], pszt[:NZ, :], AF.Exp)

                # self attention: S = q^T q (DR fp8), E = exp(S) bf16
                e_sb = wkp.tile([128, NT, NX], BF)
                zs_sb = wkp.tile([128, NT], FP)
                nc.vector.memset(zs_sb[:], 1.0)
                for mi in range(NT):
                    mw = 128 if mi < NT - 1 else LAST
                    pss_t = psb.tile([128, NX], FP, tag="big")
                    for (c0, cn) in CHUNKS:
                        nc.tensor.matmul(
                            pss_t[:, c0:c0 + cn],
                            q_sb[:, :, mi * 128:(mi + 1) * 128],
                            q_sb[:, :, c0:c0 + cn],
                            start=True, stop=True, perf_mode=DR)
                    nc.scalar.activation(e_sb[:mw, mi, :], pss_t[:mw, :], AF.Exp,
                                         accum_out=zs_sb[:mw, mi:mi + 1])
                ctx[s] = (xfg_sb, xfgp_sb, ezt_sb, e_sb, zs_sb)

            def phase2(s):
                xfg_sb, xfgp_sb, ezt_sb, e_sb, zs_sb = ctx.pop(s)

                # z tail: Z_z replicated + fast recip -> invZ bcast
                psZz = psb.tile([128, NX], FP, tag="big")
                for (c0, cn) in CHUNKS:
                    nc.tensor.matmul(psZz[:, c0:c0 + cn], ones128[:NZ, :],
                                     ezt_sb[:, c0:c0 + cn], start=True, stop=True)
                izz_sb = wkp.tile([128, NX], FP)
                nc.vector.reciprocal_approx_fast(out=izz_sb[:], in_=psZz[:])
                xemb_sb = wkp.tile([128, KT, NX], BF)
                for oi in range(KT):
                    pse = psb.tile([128, NX], FP, tag="big")
                    for (c0, cn) in CHUNKS:
                        nc.tensor.matmul(pse[:, c0:c0 + cn],
                                         zgp_sb[:, s, oi * 128:(oi + 1) * 128],
                                         ezt_sb[:, c0:c0 + cn],
                                         start=True, stop=True)
                    nc.vector.tensor_mul(xemb_sb[:, oi, :], pse[:], izz_sb[:])

                # self Z: invZ broadcast via diag tiles + ones matmul
                izs_sb = wkp.tile([128, NT], FP)
                nc.vector.reciprocal(izs_sb[:], zs_sb[:])
                diag_sb = wkp.tile([128, NT, 128], BF)
                for mi in range(NT):
                    nc.vector.tensor_scalar_mul(diag_sb[:, mi, :], ident[:],
                                                izs_sb[:, mi:mi + 1])
                psbc = psb.tile([128, NX], FP, tag="big")
                dflat = diag_sb[:].rearrange("p a b -> p (a b)")
                nc.tensor.matmul(psbc[:, 0:512], ones128[:], dflat[:, 0:512],
                                 start=True, stop=True)
                nc.tensor.matmul(psbc[:, 512:NX], ones128[:], dflat[:, 512:NX],
                                 start=True, stop=True)
                bcast_sb = wkp.tile([128, NX], FP)
                nc.vector.tensor_copy(bcast_sb[:], psbc[:])

                # self emb [c, n] = xfgp^T @ E, normalized on drain
                xself_sb = wkp.tile([128, KT, NX], BF)
                for oi in range(KT):
                    psu = psb.tile([128, NX], FP, tag="big")
                    for k in range(NT):
                        kw = 128 if k < NT - 1 else LAST
                        for (c0, cn) in CHUNKS:
                            nc.tensor.matmul(
                                psu[:, c0:c0 + cn],
                                xfgp_sb[:kw, k, oi * 128:(oi + 1) * 128],
                                e_sb[:kw, k, c0:c0 + cn],
                                start=(k == 0), stop=(k == NT - 1))
                    nc.vector.tensor_mul(xself_sb[:, oi, :], psu[:], bcast_sb[:])

                # final conv: out = relu(fis*(Wfi @ [emb; self; xfg]) + fib)
                xcat = [xemb_sb, xself_sb, xfg_sb]
                out_sb = iop.tile([128, KT, NX], FP)
                for oi in range(KT):
                    psf = psb.tile([128, NX], FP, tag="big")
                    for k in range(FKT):
                        sec, kk = divmod(k, KT)
                        for (c0, cn) in CHUNKS:
                            nc.tensor.matmul(
                                psf[:, c0:c0 + cn],
                                wfi_sb[:, k, oi * 128:(oi + 1) * 128],
                                xcat[sec][:, kk, c0:c0 + cn],
                                start=(k == 0), stop=(k == FKT - 1))
                    if nonzero_fib:
                        nc.scalar.activation(out_sb[:, oi, :], psf[:], AF.Relu,
                                             bias=fib[oi], scale=fis[oi])
                    else:
                        nc.vector.tensor_scalar(
                            out_sb[:, oi, :], psf[:], fis[oi], 0.0,
                            mybir.AluOpType.mult, mybir.AluOpType.max)
                    nc.sync.dma_start(out_d[s, oi * 128:(oi + 1) * 128, :],
                                      out_sb[:, oi, :])

            # software pipeline: overlap sample s's tail with s+1's head
            phase1(0)
            for s in range(1, BL):
                phase1(s)
                phase2(s - 1)
            phase2(BL - 1)

    nc.compile()
    return nc


_NC_CACHE = {}


def kernel(**inputs):
    xf = np.ascontiguousarray(inputs["xf"], dtype=np.float32).reshape(B, C, NX)
    zf = np.ascontiguousarray(inputs["zf"], dtype=np.float32).reshape(B, C, NZ)
    Wq = np.asarray(inputs["Wq"], dtype=np.float32)
    bq_v = np.asarray(inputs["bq"], dtype=np.float32)
    Ws = np.asarray(inputs["Ws"], dtype=np.float32)
    bs_v = np.asarray(inputs["bs"], dtype=np.float32)
    Wg = np.asarray(inputs["Wg"], dtype=np.float32)
    bg_v = np.asarray(inputs["bg"], dtype=np.float32)

    g_s = inputs["g_gamma"].astype(np.float32) / np.sqrt(inputs["g_var"].astype(np.float32) + EPS)
    g_b = (bg_v - inputs["g_mean"].astype(np.float32)) * g_s + inputs["g_beta"].astype(np.float32)
    Wg_eff = (g_s[:, None] * Wg).astype(np.float32)

    fi_s = inputs["fi_gamma"].astype(np.float32) / np.sqrt(inputs["fi_var"].astype(np.float32) + EPS)
    fi_b = ((inputs["bfi"].astype(np.float32) - inputs["fi_mean"].astype(np.float32)) * fi_s
            + inputs["fi_beta"].astype(np.float32))
    Wfi = np.asarray(inputs["Wfi"], dtype=np.float32)

    vecs = np.stack([bq_v, bs_v, g_b, fi_s, fi_b]).reshape(5, 2, 128).astype(np.float32)
    nonzero_bg = bool(np.any(g_b != 0.0))
    nonzero_fib = bool(np.any(fi_b != 0.0))

    key = (nonzero_bg, nonzero_fib)
    if key not in _NC_CACHE:
        _NC_CACHE[key] = build(*key)
    nc = _NC_CACHE[key]

    import ml_dtypes
    bf16 = ml_dtypes.bfloat16
    wqT = np.ascontiguousarray(Wq.T).astype(bf16)
    wsT = np.ascontiguousarray(Ws.T).astype(bf16)
    wgT = np.ascontiguousarray(Wg_eff.T).astype(bf16)
    wfiT = np.ascontiguousarray(Wfi.T).astype(bf16)
    xf_b = xf.astype(bf16)
    zf_b = zf.astype(bf16)

    in_maps = []
    for i in range(NCORES):
        in_maps.append({
            "xf": np.ascontiguousarray(xf_b[i * BL:(i + 1) * BL]),
            "zf": np.ascontiguousarray(zf_b[i * BL:(i + 1) * BL]),
            "wqT": wqT, "wsT": wsT, "wgT": wgT, "wfiT": wfiT,
            "vecs": vecs,
        })

    import os
    trace = os.environ.get("BASS_KERNEL_TRACE", "0") == "1"
    res = run_bass_kernel_spmd(nc, in_maps, list(range(NCORES)), trace=trace)
    LAST_RUN["exec_time_ns"] = res.exec_time_ns
    if res.instructions_and_trace is not None:
        LAST_RUN["trace_path"] = res.instructions_and_trace[1]
    LAST_RUN["profile_json"] = res.profile_json
    out = np.concatenate([r["out"] for r in res.results], axis=0)
    return out.reshape(B, O, HX, WX).astype(np.float32)


LAST_RUN = {}


if __name__ == "__main__":
    rng = np.random.default_rng(0)
    demo = {
        "zf": rng.standard_normal((B, C, HZ, WZ), dtype=np.float32),
        "xf": rng.standard_normal((B, C, HX, WX), dtype=np.float32),
        "Wq": rng.standard_normal((C, C), dtype=np.float32) * 0.02,
        "bq": np.zeros(C, np.float32),
        "Ws": rng.standard_normal((C, C), dtype=np.float32) * 0.02,
        "bs": np.zeros(C, np.float32),
        "Wg": rng.standard_normal((C, C), dtype=np.float32) * 0.02,
        "bg": np.zeros(C, np.float32),
        "g_gamma": np.ones(C, np.float32), "g_beta": np.zeros(C, np.float32),
        "g_mean": np.zeros(C, np.float32), "g_var": np.ones(C, np.float32),
        "Wfi": rng.standard_normal((O, 3 * C), dtype=np.float32) * 0.02,
        "bfi": np.zeros(O, np.float32),
        "fi_gamma": np.ones(O, np.float32), "fi_beta": np.zeros(O, np.float32),
        "fi_mean": np.zeros(O, np.float32), "fi_var": np.ones(O, np.float32),
    }
    print(kernel(**demo).shape)


# revision 22
# speedup vs baseline: 1.5881x; 1.0361x over previous
"""Trainium2 Bass kernel for Graph_Attention_Union (gnn_message_passing).

Data-parallel over batch: B=32 sharded as 4 samples per core x 8 cores.
All compute per-sample stays on one core; no collectives.

v3 design (mixed precision, measured against a 2e-2 rel-err budget):
 - Self-attention scores are diagonal-dominated (S_nn = |q_n|^2 ~ 26..44)
   so exp(S) spans e^-14..e^44: E must be bf16 (fp8 overflows -> 17% err).
   Quantizing the xfg path or the final-conv operands to fp8 adds ~4% err
   each, so all value/projection/final matmuls run bf16 (1 PE cycle/row).
 - The ONE affordable fp8 step (+0.6% err): storing q and zt in fp8e4 and
   computing the score matmuls with MatmulPerfMode.DoubleRow (K=256 packed,
   0.5 cycles/row). q/zt feed nothing but scores.
 - Self-branch Z: accum_out on the exp activations ([128,NT] row sums),
   small DVE reciprocal, identity-scaled diag tiles (Pool) and a ones
   matmul broadcast (baseline-proven pattern).
 - z-branch attention is computed TRANSPOSED (S_z^T [49-on-partitions, NX])
   so its exp is one activation instruction; Z_z comes replicated across
   partitions from an all-ones stationary matmul and reciprocal_approx_fast
   turns it into the broadcast invZ tile directly (plain reciprocal on
   [128,961] costs 6.2us -- approx_fast is ~5x faster at 18 bits).
 - Engine split: ACT = q-bias-cast + all exps (+accum) ; DVE = xfg/xfgp
   relus, norm-muls, reciprocals, final relu; Pool = diag tiles, memsets.
"""

import sys

for _p in ("/opt/trn_rl_repo",):
    if _p not in sys.path:
        sys.path.insert(0, _p)

import numpy as np

from concourse import bacc, bass, masks, mybir
from concourse.bass_utils import run_bass_kernel_spmd
from concourse.tile import TileContext

FP = mybir.dt.float32
BF = mybir.dt.bfloat16
F8 = mybir.dt.float8e4
AF = mybir.ActivationFunctionType
DR = mybir.MatmulPerfMode.DoubleRow

B, C, O = 32, 256, 256
HZ, WZ, HX, WX = 7, 7, 31, 31
NZ, NX = HZ * WZ, HX * WX  # 49, 961
NCORES = 8
BL = B // NCORES  # 4 samples per core
EPS = 1e-5

KT = C // 128           # 2 k-planes over channels
NT = (NX + 127) // 128  # 8 m-tiles over Nx (7*128 + 65)
LAST = NX - 7 * 128     # 65
FKT = 3 * C // 128      # 6 k-planes for final conv
NZP = 64                # zt stationary padded to 64 cols for DoubleRow

# free-dim chunks of NX (<=512 moving rows per matmul)
CHUNKS = [(0, 512), (512, NX - 512)]


def build(nonzero_bg: bool, nonzero_fib: bool):
    nc = bacc.Bacc(None, target_bir_lowering=False)

    xf_d = nc.declare_dram_parameter("xf", [BL, C, NX], BF, isOutput=False)
    zf_d = nc.declare_dram_parameter("zf", [BL, C, NZ], BF, isOutput=False)
    wq_d = nc.declare_dram_parameter("wqT", [C, C], BF, isOutput=False)
    ws_d = nc.declare_dram_parameter("wsT", [C, C], BF, isOutput=False)
    wg_d = nc.declare_dram_parameter("wgT", [C, C], BF, isOutput=False)
    wfi_d = nc.declare_dram_parameter("wfiT", [3 * C, O], BF, isOutput=False)
    vec_d = nc.declare_dram_parameter("vecs", [5, 2, 128], FP, isOutput=False)
    out_d = nc.declare_dram_parameter("out", [BL, O, NX], FP, isOutput=True)

    with TileContext(nc) as tc:
        with (
            tc.tile_pool(name="const", bufs=1) as constp,
            tc.tile_pool(name="io", bufs=2) as iop,
            tc.tile_pool(name="work", bufs=2) as wkp,
            tc.tile_pool(name="zbat", bufs=1) as zbp,
            tc.tile_pool(name="psbig", bufs=3, space="PSUM") as psb,
            tc.tile_pool(name="pssmall", bufs=2, space="PSUM") as pss,
        ):
            # ---- constants ----
            wq_sb = constp.tile([128, KT, C], BF)
            ws_sb = constp.tile([128, KT, C], BF)
            wg_sb = constp.tile([128, KT, C], BF)
            wfi_sb = constp.tile([128, FKT, O], BF)
            for k in range(KT):
                nc.scalar.dma_start(wq_sb[:, k, :], wq_d[k * 128:(k + 1) * 128, :])
                nc.sync.dma_start(ws_sb[:, k, :], ws_d[k * 128:(k + 1) * 128, :])
                nc.scalar.dma_start(wg_sb[:, k, :], wg_d[k * 128:(k + 1) * 128, :])
            for k in range(FKT):
                nc.gpsimd.dma_start(wfi_sb[:, k, :], wfi_d[k * 128:(k + 1) * 128, :])
            vecs = constp.tile([128, 5, 2], FP)
            nc.sync.dma_start(vecs[:], vec_d.rearrange("v t p -> p v t"))
            bq = [vecs[:, 0, t:t + 1] for t in range(2)]
            bs = [vecs[:, 1, t:t + 1] for t in range(2)]
            bg = [vecs[:, 2, t:t + 1] for t in range(2)]
            fis = [vecs[:, 3, t:t + 1] for t in range(2)]
            fib = [vecs[:, 4, t:t + 1] for t in range(2)]
            bg_row = constp.tile([1, C], FP)
            nc.gpsimd.dma_start(bg_row[:], vec_d[2:3].rearrange("o t p -> o (t p)"))
            ones_row = constp.tile([1, 128], FP)
            nc.vector.memset(ones_row[:], 1.0)
            ones128 = constp.tile([128, 128], BF)
            nc.vector.memset(ones128[:], 1.0)
            ident = constp.tile([128, 128], BF)
            masks.make_identity(nc, ident[:])

            # ---- z-branch convs, batched over all samples (tiny) ----
            zf_sb = zbp.tile([128, KT, BL, NZ], BF)
            for k in range(KT):
                nc.sync.dma_start(
                    zf_sb[:, k, :, :],
                    zf_d[:, k * 128:(k + 1) * 128, :].rearrange("s c n -> c s n"))
            # zt: fp8, padded to 64 cols per sample for the DR score matmul
            zt_sb = zbp.tile([128, KT, BL, NZP], F8)
            nc.gpsimd.memset(zt_sb[:, :, :, NZ:], 0.0)
            zg_sb = zbp.tile([128, KT, BL, NZ], BF)
            for oi in range(KT):
                psz = pss.tile([128, BL, NZ], FP, tag="small")
                for k in range(KT):
                    nc.tensor.matmul(
                        psz[:].rearrange("p s n -> p (s n)"),
                        ws_sb[:, k, oi * 128:(oi + 1) * 128],
                        zf_sb[:, k, :, :].rearrange("p s n -> p (s n)"),
                        start=(k == 0), stop=(k == KT - 1))
                nc.scalar.activation(
                    zt_sb[:, oi, :, :NZ], psz[:],
                    AF.Identity, bias=bs[oi])
                psz2 = pss.tile([128, BL * NZ], FP, tag="small")
                for k in range(KT):
                    nc.tensor.matmul(
                        psz2[:], wg_sb[:, k, oi * 128:(oi + 1) * 128],
                        zf_sb[:, k, :, :].rearrange("p s n -> p (s n)"),
                        start=(k == 0), stop=(k == KT - 1))
                nc.scalar.activation(
                    zg_sb[:, oi, :, :].rearrange("p s n -> p (s n)"), psz2[:],
                    AF.Relu, bias=bg[oi])
            # zgp [m=49, s, c] = zg^T
            zgp_sb = zbp.tile([NZ, BL, C], BF)
            for s in range(BL):
                for oi in range(KT):
                    pst = pss.tile([NZ, 128], BF, tag="small")
                    nc.tensor.transpose(pst[:], zg_sb[:, oi, s, :], ident[:])
                    nc.vector.tensor_copy(zgp_sb[:, s, oi * 128:(oi + 1) * 128], pst[:])

            ctx = {}

            def phase1(s):
                # loads + projections + scores + exps
                xf_sb = iop.tile([128, KT, NX], BF)
                for k in range(KT):
                    for (c0, cn) in CHUNKS:
                        nc.sync.dma_start(xf_sb[:, k, c0:c0 + cn],
                                          xf_d[s, k * 128:(k + 1) * 128, c0:c0 + cn])

                # q: bf16 conv -> fp8 store (padded cols for DR)
                q_sb = wkp.tile([128, KT, NT * 128], F8)
                nc.gpsimd.memset(q_sb[:, :, NX:], 0.0)
                for oi in range(KT):
                    psq = psb.tile([128, NX], FP, tag="big")
                    for k in range(KT):
                        for (c0, cn) in CHUNKS:
                            nc.tensor.matmul(
                                psq[:, c0:c0 + cn],
                                wq_sb[:, k, oi * 128:(oi + 1) * 128],
                                xf_sb[:, k, c0:c0 + cn],
                                start=(k == 0), stop=(k == KT - 1))
                    nc.scalar.activation(q_sb[:, oi, :NX], psq[:], AF.Identity,
                                         bias=bq[oi])

                # xfg: bf16 conv, relu (DVE)
                xfg_sb = wkp.tile([128, KT, NX], BF)
                for oi in range(KT):
                    psg = psb.tile([128, NX], FP, tag="big")
                    for k in range(KT):
                        for (c0, cn) in CHUNKS:
                            nc.tensor.matmul(
                                psg[:, c0:c0 + cn],
                                wg_sb[:, k, oi * 128:(oi + 1) * 128],
                                xf_sb[:, k, c0:c0 + cn],
                                start=(k == 0), stop=(k == KT - 1))
                    nc.vector.tensor_scalar(
                        xfg_sb[:, oi, :], psg[:], bg[oi], 0.0,
                        mybir.AluOpType.add, mybir.AluOpType.max)

                # xfgp [n, c]: duplicate conv, relu (DVE)
                xfgp_sb = wkp.tile([128, NT, C], BF)
                for mi in range(NT):
                    mw = 128 if mi < NT - 1 else LAST
                    psp = pss.tile([128, C], FP, tag="small")
                    for k in range(KT):
                        nc.tensor.matmul(
                            psp[:mw, :],
                            xf_sb[:, k, mi * 128:mi * 128 + mw],
                            wg_sb[:, k, :],
                            start=(k == 0),
                            stop=(k == KT - 1) and not nonzero_bg)
                    if nonzero_bg:
                        nc.tensor.matmul(psp[:mw, :], ones_row[:, :mw], bg_row[:],
                                         start=False, stop=True)
                    nc.vector.tensor_scalar_max(xfgp_sb[:mw, mi, :], psp[:mw, :], 0.0)

                # self attention: S = q^T q (DR fp8), E = exp(S) bf16
                e_sb = wkp.tile([128, NT, NX], BF)
                zs_sb = wkp.tile([128, NT], FP)
                nc.vector.memset(zs_sb[:], 1.0)
                for mi in range(NT):
                    mw = 128 if mi < NT - 1 else LAST
                    pss_t = psb.tile([128, NX], FP, tag="big")
                    for (c0, cn) in CHUNKS:
                        nc.tensor.matmul(
                            pss_t[:, c0:c0 + cn],
                            q_sb[:, :, mi * 128:(mi + 1) * 128],
                            q_sb[:, :, c0:c0 + cn],
                            start=True, stop=True, perf_mode=DR)
                    nc.scalar.activation(e_sb[:mw, mi, :], pss_t[:mw, :], AF.Exp,
                                         accum_out=zs_sb[:mw, mi:mi + 1])
                ctx[s] = [xfg_sb, xfgp_sb, q_sb, e_sb, zs_sb]

            def phase1b(s):
                # z attention, transposed: S_z^T (DR fp8, 64 padded)
                q_sb = ctx[s][2]
                ezt_sb = wkp.tile([NZ, NX], BF)
                pszt = psb.tile([NZP, NX], FP, tag="big")
                for (c0, cn) in CHUNKS:
                    nc.tensor.matmul(
                        pszt[:, c0:c0 + cn],
                        zt_sb[:, :, s, :],
                        q_sb[:, :, c0:c0 + cn],
                        start=True, stop=True, perf_mode=DR)
                nc.scalar.activation(ezt_sb[:], pszt[:NZ, :], AF.Exp)
                ctx[s][2] = ezt_sb

            def phase2(s):
                xfg_sb, xfgp_sb, ezt_sb, e_sb, zs_sb = ctx.pop(s)

                # z tail: Z_z replicated + fast recip -> invZ bcast
                psZz = psb.tile([128, NX], FP, tag="big")
                for (c0, cn) in CHUNKS:
                    nc.tensor.matmul(psZz[:, c0:c0 + cn], ones128[:NZ, :],
                                     ezt_sb[:, c0:c0 + cn], start=True, stop=True)
                izz_sb = wkp.tile([128, NX], FP)
                nc.vector.reciprocal_approx_fast(out=izz_sb[:], in_=psZz[:])
                xemb_sb = wkp.tile([128, KT, NX], BF)
                for oi in range(KT):
                    pse = psb.tile([128, NX], FP, tag="big")
                    for (c0, cn) in CHUNKS:
                        nc.tensor.matmul(pse[:, c0:c0 + cn],
                                         zgp_sb[:, s, oi * 128:(oi + 1) * 128],
                                         ezt_sb[:, c0:c0 + cn],
                                         start=True, stop=True)
                    nc.vector.tensor_mul(xemb_sb[:, oi, :], pse[:], izz_sb[:])

                # self Z: invZ broadcast via diag tiles + ones matmul
                izs_sb = wkp.tile([128, NT], FP)
                nc.vector.reciprocal(izs_sb[:], zs_sb[:])
                diag_sb = wkp.tile([128, NT, 128], BF)
                for mi in range(NT):
                    nc.vector.tensor_scalar_mul(diag_sb[:, mi, :], ident[:],
                                                izs_sb[:, mi:mi + 1])
                psbc = psb.tile([128, NX], FP, tag="big")
                dflat = diag_sb[:].rearrange("p a b -> p (a b)")
                nc.tensor.matmul(psbc[:, 0:512], ones128[:], dflat[:, 0:512],
                                 start=True, stop=True)
                nc.tensor.matmul(psbc[:, 512:NX], ones128[:], dflat[:, 512:NX],
                                 start=True, stop=True)
                bcast_sb = wkp.tile([128, NX], FP)
                nc.vector.tensor_copy(bcast_sb[:], psbc[:])

                # self emb [c, n] = xfgp^T @ E, normalized on drain
                xself_sb = wkp.tile([128, KT, NX], BF)
                for oi in range(KT):
                    psu = psb.tile([128, NX], FP, tag="big")
                    for k in range(NT):
                        kw = 128 if k < NT - 1 else LAST
                        for (c0, cn) in CHUNKS:
                            nc.tensor.matmul(
                                psu[:, c0:c0 + cn],
                                xfgp_sb[:kw, k, oi * 128:(oi + 1) * 128],
                                e_sb[:kw, k, c0:c0 + cn],
                                start=(k == 0), stop=(k == NT - 1))
                    nc.vector.tensor_mul(xself_sb[:, oi, :], psu[:], bcast_sb[:])

                # final conv: out = relu(fis*(Wfi @ [emb; self; xfg]) + fib)
                xcat = [xemb_sb, xself_sb, xfg_sb]
                out_sb = iop.tile([128, KT, NX], FP)
                for oi in range(KT):
                    psf = psb.tile([128, NX], FP, tag="big")
                    for (c0, cn) in CHUNKS:
                        for k in range(FKT):
                            sec, kk = divmod(k, KT)
                            nc.tensor.matmul(
                                psf[:, c0:c0 + cn],
                                wfi_sb[:, k, oi * 128:(oi + 1) * 128],
                                xcat[sec][:, kk, c0:c0 + cn],
                                start=(k == 0), stop=(k == FKT - 1))
                        if nonzero_fib:
                            nc.scalar.activation(out_sb[:, oi, c0:c0 + cn],
                                                 psf[:, c0:c0 + cn], AF.Relu,
                                                 bias=fib[oi], scale=fis[oi])
                        else:
                            nc.vector.tensor_scalar(
                                out_sb[:, oi, c0:c0 + cn], psf[:, c0:c0 + cn],
                                fis[oi], 0.0,
                                mybir.AluOpType.mult, mybir.AluOpType.max)
                        nc.sync.dma_start(
                            out_d[s, oi * 128:(oi + 1) * 128, c0:c0 + cn],
                            out_sb[:, oi, c0:c0 + cn])

            # software pipeline: overlap sample s's tail with s+1's head
            phase1(0)
            phase1b(0)
            for s in range(1, BL):
                phase1(s)
                phase1b(s)
                phase2(s - 1)
            phase2(BL - 1)

    nc.compile()
    return nc


_NC_CACHE = {}


def kernel(**inputs):
    xf = np.ascontiguousarray(inputs["xf"], dtype=np.float32).reshape(B, C, NX)
    zf = np.ascontiguousarray(inputs["zf"], dtype=np.float32).reshape(B, C, NZ)
    Wq = np.asarray(inputs["Wq"], dtype=np.float32)
    bq_v = np.asarray(inputs["bq"], dtype=np.float32)
    Ws = np.asarray(inputs["Ws"], dtype=np.float32)
    bs_v = np.asarray(inputs["bs"], dtype=np.float32)
    Wg = np.asarray(inputs["Wg"], dtype=np.float32)
    bg_v = np.asarray(inputs["bg"], dtype=np.float32)

    g_s = inputs["g_gamma"].astype(np.float32) / np.sqrt(inputs["g_var"].astype(np.float32) + EPS)
    g_b = (bg_v - inputs["g_mean"].astype(np.float32)) * g_s + inputs["g_beta"].astype(np.float32)
    Wg_eff = (g_s[:, None] * Wg).astype(np.float32)

    fi_s = inputs["fi_gamma"].astype(np.float32) / np.sqrt(inputs["fi_var"].astype(np.float32) + EPS)
    fi_b = ((inputs["bfi"].astype(np.float32) - inputs["fi_mean"].astype(np.float32)) * fi_s
            + inputs["fi_beta"].astype(np.float32))
    Wfi = np.asarray(inputs["Wfi"], dtype=np.float32)

    vecs = np.stack([bq_v, bs_v, g_b, fi_s, fi_b]).reshape(5, 2, 128).astype(np.float32)
    nonzero_bg = bool(np.any(g_b != 0.0))
    nonzero_fib = bool(np.any(fi_b != 0.0))

    key = (nonzero_bg, nonzero_fib)
    if key not in _NC_CACHE:
        _NC_CACHE[key] = build(*key)
    nc = _NC_CACHE[key]

    import ml_dtypes
    bf16 = ml_dtypes.bfloat16
    wqT = np.ascontiguousarray(Wq.T).astype(bf16)
    wsT = np.ascontiguousarray(Ws.T).astype(bf16)
    wgT = np.ascontiguousarray(Wg_eff.T).astype(bf16)
    wfiT = np.ascontiguousarray(Wfi.T).astype(bf16)
    xf_b = xf.astype(bf16)
    zf_b = zf.astype(bf16)

    in_maps = []
    for i in range(NCORES):
        in_maps.append({
            "xf": np.ascontiguousarray(xf_b[i * BL:(i + 1) * BL]),
            "zf": np.ascontiguousarray(zf_b[i * BL:(i + 1) * BL]),
            "wqT": wqT, "wsT": wsT, "wgT": wgT, "wfiT": wfiT,
            "vecs": vecs,
        })

    import os
    trace = os.environ.get("BASS_KERNEL_TRACE", "0") == "1"
    res = run_bass_kernel_spmd(nc, in_maps, list(range(NCORES)), trace=trace)
    LAST_RUN["exec_time_ns"] = res.exec_time_ns
    if res.instructions_and_trace is not None:
        LAST_RUN["trace_path"] = res.instructions_and_trace[1]
    LAST_RUN["profile_json"] = res.profile_json
    out = np.concatenate([r["out"] for r in res.results], axis=0)
    return out.reshape(B, O, HX, WX).astype(np.float32)


LAST_RUN = {}


if __name__ == "__main__":
    rng = np.random.default_rng(0)
    demo = {
        "zf": rng.standard_normal((B, C, HZ, WZ), dtype=np.float32),
        "xf": rng.standard_normal((B, C, HX, WX), dtype=np.float32),
        "Wq": rng.standard_normal((C, C), dtype=np.float32) * 0.02,
        "bq": np.zeros(C, np.float32),
        "Ws": rng.standard_normal((C, C), dtype=np.float32) * 0.02,
        "bs": np.zeros(C, np.float32),
        "Wg": rng.standard_normal((C, C), dtype=np.float32) * 0.02,
        "bg": np.zeros(C, np.float32),
        "g_gamma": np.ones(C, np.float32), "g_beta": np.zeros(C, np.float32),
        "g_mean": np.zeros(C, np.float32), "g_var": np.ones(C, np.float32),
        "Wfi": rng.standard_normal((O, 3 * C), dtype=np.float32) * 0.02,
        "bfi": np.zeros(O, np.float32),
        "fi_gamma": np.ones(O, np.float32), "fi_beta": np.zeros(O, np.float32),
        "fi_mean": np.zeros(O, np.float32), "fi_var": np.ones(O, np.float32),
    }
    print(kernel(**demo).shape)
